# revision 1
# baseline (speedup 1.0000x reference)
"""BitLinear forward on 8 TRN2 NeuronCores (tensor-parallel, column-parallel linear).

  alpha = mean(|W|)            (scalar over the FULL weight matrix)
  y     = x @ (sign(W) * alpha)^T

Sharding: W rows (out_features) split across 8 cores; x replicated; each core
computes y[:, c*2048:(c+1)*2048]; alpha = local |W| reduction + AllReduce.

Per-core device pipeline:
  1. W pass: load W shard fp32, sign()->bf16 scratch in DRAM, abs-row-sums.
  2. alpha: DVE reduce -> gpsimd partition_all_reduce -> AllReduce(8 cores) -> scale.
  3. WT: XBAR DMA-transpose-load sign(W) -> SBUF [128, 32, 2048], cast fp8e4
     (+-1 exact; mixed bf16 x fp8 matmul runs at bf16 rate, halves SBUF).
  4. Per 128-row x tile: load fp32 -> cast bf16 -> DRAM scratch -> DMA-transpose
     -> xT [128, 32, 128]; 32x4 matmuls accumulate [128, 2048] fp32 in PSUM;
     ScalarE Copy*alpha eviction; DMA out.

Matmul mapping: out[s, o] += xT[i, s].T @ WT[i, o]  (K=i on partitions).
"""
import sys
import os

sys.path.insert(0, "/opt/trn_rl_repo")
import numpy as np

P = 128
S, I, O = 8192, 4096, 16384
N_CORES = 8
OC = O // N_CORES          # 2048 out-features per core
KB = I // P                # 32 contraction blocks
NT = S // P                # 64 x row-tiles
NJ = OC // 512             # 4 psum bank chunks

_cache = {}


def _build():
    from concourse import bacc, tile, mybir, bass_isa

    dt = mybir.dt
    nc = bacc.Bacc("TRN2", target_bir_lowering=False, debug=False, num_devices=N_CORES)
    x_ap = nc.dram_tensor("x", [S, I], dt.float32, kind="ExternalInput").ap()
    w_ap = nc.dram_tensor("w", [OC, I], dt.float32, kind="ExternalInput").ap()
    y_ap = nc.dram_tensor("y", [S, OC], dt.float32, kind="ExternalOutput").ap()

    with tile.TileContext(nc) as tc:
        with (
            tc.tile_pool(name="pers", bufs=1) as pers,
            tc.tile_pool(name="ld32", bufs=2) as ld32,
            tc.tile_pool(name="s16", bufs=3) as s16,
            tc.tile_pool(name="wtmp", bufs=2) as wtmp,
            tc.tile_pool(name="pxT", bufs=3) as pxT,
            tc.tile_pool(name="pyo", bufs=2) as pyo,
            tc.tile_pool(name="psum", bufs=2, space="PSUM") as psum,
            tc.tile_pool(name="dramw", bufs=1, space="DRAM") as dramw,
            tc.tile_pool(name="dramx", bufs=NT, space="DRAM") as dramx,
            tc.tile_pool(name="dramc", bufs=1, space="DRAM") as dramc,
        ):
            # ---- W pass: sign -> bf16 scratch, |w| partial sums
            wsgn = dramw.tile([OC, I], dt.bfloat16)
            wabs = pers.tile([P, OC // P], dt.float32)
            for t in range(OC // P):
                w32 = ld32.tile([P, I], dt.float32, tag="ld32")
                nc.sync.dma_start(w32[:], w_ap[t * P:(t + 1) * P, :])
                sg = s16.tile([P, I], dt.bfloat16, tag="s16")
                nc.scalar.sign(sg[:], w32[:])
                nc.sync.dma_start(wsgn[t * P:(t + 1) * P, :], sg[:])
                nc.vector.tensor_reduce(
                    wabs[:, t:t + 1], w32[:], axis=mybir.AxisListType.XYZW,
                    op=mybir.AluOpType.add, apply_absolute_value=True)

            # ---- alpha: local reduce -> partition allreduce -> 8-core AllReduce
            wsum = pers.tile([P, 1], dt.float32)
            nc.vector.tensor_reduce(
                wsum[:], wabs[:], axis=mybir.AxisListType.XYZW,
                op=mybir.AluOpType.add)
            par = pers.tile([P, 1], dt.float32)
            nc.gpsimd.partition_all_reduce(
                par[:], wsum[:], channels=P, reduce_op=bass_isa.ReduceOp.add)
            cc_in = dramc.tile([P, 1], dt.float32)
            cc_out = dramc.tile([P, 1], dt.float32, addr_space="Shared")
            nc.sync.dma_start(cc_in[:], par[:])
            nc.gpsimd.collective_compute(
                "AllReduce", mybir.AluOpType.add,
                replica_groups=[list(range(N_CORES))],
                ins=[cc_in[:].opt()], outs=[cc_out[:].opt()])
            asum = pers.tile([P, 1], dt.float32)
            nc.sync.dma_start(asum[:], cc_out[:])
            alpha = pers.tile([P, 1], dt.float32)
            nc.vector.tensor_scalar_mul(alpha[:], asum[:], 1.0 / (float(O) * float(I)))

            # ---- WT: transpose-load sign(W) and cast to fp8 (+-1 exact)
            WT = pers.tile([P, KB, OC], dt.float8e4)
            for c in range(16):
                tmp = wtmp.tile([P, 2, OC], dt.bfloat16, tag="wtmp")
                nc.sync.dma_start_transpose(tmp[:], wsgn[:, c * 256:(c + 1) * 256])
                nc.vector.tensor_copy(WT[:, 2 * c:2 * c + 2, :], tmp[:])

            # ---- main loop over x row-tiles
            for st in range(NT):
                x32 = ld32.tile([P, I], dt.float32, tag="ld32")
                nc.sync.dma_start(x32[:], x_ap[st * P:(st + 1) * P, :])
                xc = s16.tile([P, I], dt.bfloat16, tag="s16")
                nc.vector.tensor_copy(xc[:], x32[:])
                xb = dramx.tile([P, I], dt.bfloat16, tag="xb")
                nc.sync.dma_start(xb[:], xc[:])
                xT = pxT.tile([P, KB, P], dt.bfloat16, tag="xT")
                nc.sync.dma_start_transpose(xT[:], xb[:])
                ps = psum.tile([P, OC], dt.float32, tag="ps")
                for k in range(KB):
                    for j in range(NJ):
                        nc.tensor.matmul(
                            ps[:, j * 512:(j + 1) * 512],
                            xT[:, k, :],
                            WT[:, k, j * 512:(j + 1) * 512],
                            start=(k == 0), stop=(k == KB - 1))
                yo = pyo.tile([P, OC], dt.float32, tag="yo")
                nc.scalar.activation(
                    yo[:], ps[:], mybir.ActivationFunctionType.Copy,
                    bias=0.0, scale=alpha[:, 0:1])
                nc.sync.dma_start(y_ap[st * P:(st + 1) * P, :], yo[:])

    nc.compile()
    return nc


def _get_nc():
    if "nc" not in _cache:
        _cache["nc"] = _build()
    return _cache["nc"]


def kernel(x: np.ndarray, weight: np.ndarray) -> np.ndarray:
    from concourse.bass_utils import run_bass_kernel_spmd

    nc = _get_nc()
    xf = np.ascontiguousarray(np.asarray(x, dtype=np.float32).reshape(S, I))
    wf = np.asarray(weight, dtype=np.float32)
    in_maps = [
        {"x": xf, "w": np.ascontiguousarray(wf[c * OC:(c + 1) * OC])}
        for c in range(N_CORES)
    ]
    res = run_bass_kernel_spmd(
        nc, in_maps, core_ids=list(range(N_CORES)),
        trace=bool(int(os.environ.get("BITLINEAR_TRACE", "0"))),
    )
    if res.exec_time_ns is not None:
        _cache["exec_time_ns"] = res.exec_time_ns
    _cache["last_results"] = res
    y = np.concatenate([res.results[c]["y"] for c in range(N_CORES)], axis=1)
    return y.reshape(2, S // 2, O)
